# revision 7
# baseline (speedup 1.0000x reference)
"""DTNetv0 forward kernel for 8 Trainium2 NeuronCores.

Computes, for x [B,128], W1 [511,128], b1 [511], W2 [512,1022],
leaf_actions [512] (32 leaves per each of 16 actions):

    h = x @ W1.T + b1
    z = [relu(h), relu(-h)]
    y = z @ W2.T
    pooled[b,a] = max over leaves l with action a of y[b,l]
    out = softmax(pooled, axis=-1)

Sharding: pure data parallelism — batch split 8 ways, weights replicated.

Algebraic fold (the key PE-cycle saver): with W2 = [W2a | W2b] split at the
relu(h)/relu(-h) boundary and relu(h) = (h+|h|)/2, relu(-h) = (|h|-h)/2,

    y = h @ A.T + |h| @ B.T,   A = (W2a - W2b)/2,  B = (W2a + W2b)/2
      = x @ C.T + |h| @ B.T + c0,   C = A @ W1,  c0 = A @ b1

so the 1022-wide second contraction becomes a 511-wide one (|h| @ B.T) plus
a 128-wide one (x @ C.T). c0 rides for free through the padded node row:
w1t column 511 is zero and b1 pad is 1.0, so |h|[511] == 1 exactly, and
B.T row 511 holds c0.

ALL matmul operands are bf16: measured on this silicon, bf16 streams at
1 col/cycle with stationary loads fully hidden, while float32r streams at
~1.5 cy/col — bf16 cuts PE time by a third at rel-err 4.4e-3 (gate 2e-2).
x is pre-transposed AND pre-cast to bf16 on the host, halving its DMA.

Per 512-row batch tile, on device:
    hT [512nodes,512b] linear1: 4 bf16 matmuls (W1T stationary, xt moving)
    aT [512, 512b]     Abs(h + b1) on ACT, PSUM -> bf16 SBUF
    y  [128b, 512lv]   linear2 BATCH-MAJOR per 128-batch subtile: 1 matmul
                       x-chunk stationary vs C.T moving, then 4 accumulating
                       matmuls a-chunk stationary vs B.T moving
    pooled [128b, 16]  leaves are host-permuted ACTION-MAJOR (slot a*32+j),
                       so the segment max is one DVE reduce with contiguous
                       32-element runs (strided reads measured 2x slower)
    out                softmax without max-subtraction (|pooled| < 16 so
                       exp is safe in fp32): ACT Exp with accum_out for the
                       denominator, DVE reciprocal, ACT Copy-with-scale

Three-stage software pipeline in emission order: front_a (x -> aT) runs
three tiles ahead of front_b (linear2 + pooled reduce), and the softmax
tail trails one tile behind. PSUM is split 4+4: four banks for h tiles,
four rotating banks for y. Steady state 12288 PE cycles/tile = 5.12 us,
PE-bound; ACT ~1.5 us, DVE ~2.6 us, x-DMA ~0.4 us ride underneath.
"""

import numpy as np
import ml_dtypes

B, IN_DIM, N_NODES, N_LEAVES, N_ACTIONS = 131072, 128, 511, 512, 16
N_CORES = 8
B_SHARD = B // N_CORES          # 16384 rows per core
B_TILE = 512                    # batch columns per tile (one PSUM bank of fp32)
N_TILES = B_SHARD // B_TILE     # 32
NODES_P = 512                   # nodes padded 511 -> 512 (4 chunks of 128)
PER_ACTION = N_LEAVES // N_ACTIONS  # 32 leaves per action

_compiled = None  # traced+compiled Bass module cache (one per process)


def _build_nc(n_passes=1):
    import concourse.tile as tile
    from concourse import bacc, bass, mybir
    from contextlib import ExitStack

    fp32 = mybir.dt.float32
    bf16 = mybir.dt.bfloat16
    AF = mybir.ActivationFunctionType

    nc = bacc.Bacc()
    xt_h = nc.declare_dram_parameter("xt", [IN_DIM, B_SHARD], bf16, isOutput=False)
    w1t_h = nc.declare_dram_parameter("w1t", [IN_DIM, NODES_P], bf16, isOutput=False)
    b1c_h = nc.declare_dram_parameter("b1c", [128, 4], fp32, isOutput=False)
    bt_h = nc.declare_dram_parameter("bt", [128, 4, N_LEAVES], bf16, isOutput=False)
    ct_h = nc.declare_dram_parameter("ct", [IN_DIM, N_LEAVES], bf16, isOutput=False)
    out_h = nc.declare_dram_parameter("out", [B_SHARD, N_ACTIONS], fp32, isOutput=True)

    LEAD = 3          # front_a runs this many tiles ahead of front_b
    G_TILES = 8       # batch tiles per x DMA granule
    N_GRAN = N_TILES // G_TILES

    with tile.TileContext(nc) as tc, ExitStack() as ctx:
        consts = ctx.enter_context(tc.tile_pool(name="consts", bufs=1))
        xin = ctx.enter_context(tc.tile_pool(name="xin", bufs=2))
        ap = ctx.enter_context(tc.tile_pool(name="ap", bufs=4))
        sm = ctx.enter_context(tc.tile_pool(name="sm", bufs=2))
        psA = ctx.enter_context(tc.tile_pool(name="psA", bufs=4, space="PSUM"))
        # psY tiles span TWO PSUM banks so the pooled segment-max runs as one
        # DVE reduce per subtile PAIR (fewer, larger DVE instructions)
        psY = ctx.enter_context(tc.tile_pool(name="psY", bufs=2, space="PSUM"))

        # x streams in 8-tile granules (8 KiB/partition each): 4 dma_starts
        # per pass instead of 32, double-buffered one granule ahead.
        granules = {}

        def load_gran(g):
            gx = xin.tile([128, G_TILES * B_TILE], bf16, tag="xg")
            lo = g * G_TILES * B_TILE
            nc.sync.dma_start(out=gx, in_=xt_h[:, lo : lo + G_TILES * B_TILE])
            granules[g] = gx

        load_gran(0)
        b1_sb = consts.tile([128, 4], fp32)
        nc.sync.dma_start(out=b1_sb, in_=b1c_h[:, :])
        w1t_sb = consts.tile([128, NODES_P], bf16)
        nc.sync.dma_start(out=w1t_sb, in_=w1t_h[:, :])
        ct_sb = consts.tile([128, N_LEAVES], bf16)
        nc.sync.dma_start(out=ct_sb, in_=ct_h[:, :])
        # the big bt DMA rides the Activation HWDGE queue so granule loads
        # (SP queue) are not stuck behind it
        bt_sb = consts.tile([128, 4, N_LEAVES], bf16)
        nc.scalar.dma_start(out=bt_sb, in_=bt_h[:, :, :])

        def front_a(t, gi, total):
            rows = slice(t * B_TILE, (t + 1) * B_TILE)
            g, r = divmod(t, G_TILES)
            # prefetch the next granule once all readers of the granule two
            # slots back have been emitted (r == LEAD aligns exactly)
            if r == LEAD and (gi - LEAD + G_TILES) < total:
                load_gran((g + 1) % N_GRAN)
            x_sb = granules[g][:, r * B_TILE : (r + 1) * B_TILE]

            # ---- linear1 + fused bias/abs into aT [128, 4, 512] bf16 ----
            a_sb = ap.tile([128, 4, B_TILE], bf16, tag="a")
            for c in range(4):
                h_ps = psA.tile([128, B_TILE], fp32, tag="h")
                nc.tensor.matmul(
                    h_ps,
                    lhsT=w1t_sb[:, c * 128 : (c + 1) * 128],
                    rhs=x_sb,
                    start=True,
                    stop=True,
                )
                nc.scalar.activation(
                    out=a_sb[:, c, :], in_=h_ps, func=AF.Abs,
                    bias=b1_sb[:, c : c + 1], scale=1.0,
                )
            return rows, x_sb, a_sb

        def front_b(rows, x_sb, a_sb, last=False):
            # ---- linear2, batch-major: y_s [128 batch-sub, 512 leaves] ----
            pl = sm.tile([128, 4, N_ACTIONS], fp32, tag="pl")
            for g in range(2):          # subtile pairs (two PSUM banks each)
                y_ps = psY.tile([128, 2, B_TILE], fp32, tag="y")
                for i in range(2):
                    s = 2 * g + i
                    nc.tensor.matmul(
                        y_ps[:, i, :],
                        lhsT=x_sb[:, s * 128 : (s + 1) * 128],
                        rhs=ct_sb,
                        start=True,
                        stop=False,
                    )
                    for c in range(4):
                        nc.tensor.matmul(
                            y_ps[:, i, :],
                            lhsT=a_sb[:, c, s * 128 : (s + 1) * 128],
                            rhs=bt_sb[:, c, :],
                            start=False,
                            stop=(c == 3),
                        )
                # leaves are action-major (slot a*32 + j): contiguous reduce
                # over both banks in one DVE instruction
                nc.vector.tensor_reduce(
                    out=pl[:, 2 * g : 2 * g + 2, :],
                    in_=y_ps.rearrange("p i (a j) -> p i a j", a=N_ACTIONS),
                    axis=mybir.AxisListType.X,
                    op=mybir.AluOpType.max,
                )
            if last:
                tail(rows, pl)
                return None
            return rows, pl

        def tail(rows, pl):
            # ---- softmax, batch-major [128, 4, 16], no max-subtraction
            # (|pooled| is small enough that exp is safe in fp32).
            # One big ACT Exp, then a DVE-local sum -> reciprocal ->
            # broadcast-multiply chain (no cross-engine ping-pong). ----
            e = sm.tile([128, 4, N_ACTIONS], fp32, tag="e")
            nc.scalar.activation(out=e, in_=pl, func=AF.Exp, scale=1.0)
            ssum = sm.tile([128, 4], fp32, tag="ssum")
            nc.vector.tensor_reduce(
                out=ssum, in_=e, axis=mybir.AxisListType.X,
                op=mybir.AluOpType.add,
            )
            rcp = sm.tile([128, 4], fp32, tag="rcp")
            nc.vector.reciprocal(rcp, ssum)
            o = sm.tile([128, 4, N_ACTIONS], fp32, tag="o")
            rcp_ap = rcp[:, :]
            rcp_bc = bass.AP(rcp_ap.tensor, rcp_ap.offset,
                             rcp_ap.ap + [[0, N_ACTIONS]])
            nc.vector.tensor_tensor(out=o, in0=e, in1=rcp_bc,
                                    op=mybir.AluOpType.mult)
            nc.sync.dma_start(
                out=out_h[rows, :].rearrange("(s p) a -> p s a", p=128), in_=o
            )

        back = tail

        # 3-deep software pipeline: front_a (x -> aT) runs three tiles ahead
        # of front_b (linear2 + pooled reduce); back trails one tile behind.
        total = N_TILES * n_passes
        fa = []
        for t in range(min(LEAD, total)):
            fa.append(front_a(t, t, total))
        pending = None
        for i in range(total):
            cur = front_b(*fa.pop(0), last=(i == total - 1))
            if i + LEAD < total:
                fa.append(front_a((i + LEAD) % N_TILES, i + LEAD, total))
            if pending is not None:
                back(*pending)
            pending = cur
        if pending is not None:
            back(*pending)

    nc.compile()
    return nc


def _prep_weights(W1, b1, W2, leaf_actions):
    """Host-side weight prep: fold the linear half of the relu pair into x
    (C = A@W1, c0 = A@b1) and keep only the |h| half (B) at full width.
    Leaves are permuted ACTION-MAJOR: slot a*32+j holds the j-th leaf of
    action a, so the on-device segment-max reads contiguous runs."""
    W1 = np.asarray(W1, np.float64)
    b1 = np.asarray(b1, np.float64)
    W2 = np.asarray(W2, np.float64)
    bf = ml_dtypes.bfloat16

    la = np.asarray(leaf_actions).astype(np.int64)
    perm = np.empty(N_LEAVES, np.int64)
    for a in range(N_ACTIONS):
        (grp,) = np.nonzero(la == a)
        assert len(grp) == PER_ACTION, "kernel assumes 32 leaves per action"
        perm[a * PER_ACTION + np.arange(PER_ACTION)] = grp

    W2p = W2[perm]                              # [512, 1022] leaf-permuted
    Am = (W2p[:, :N_NODES] - W2p[:, N_NODES:]) * 0.5   # [512, 511]
    Bm = (W2p[:, :N_NODES] + W2p[:, N_NODES:]) * 0.5   # [512, 511]
    C = Am @ W1                                 # [512, 128]
    c0 = Am @ b1                                # [512]

    w1t = np.zeros((IN_DIM, NODES_P), np.float32)
    w1t[:, :N_NODES] = W1.T                     # col 511 stays zero
    b1c = np.zeros((4, 128), np.float32)
    b1c.reshape(-1)[:N_NODES] = b1
    b1c.reshape(-1)[N_NODES] = 1.0              # pad node: |h|[511] == 1
    b1c = np.ascontiguousarray(b1c.T)           # [128, 4]

    btm = np.zeros((NODES_P, N_LEAVES), np.float32)
    btm[:N_NODES, :] = Bm.T
    btm[N_NODES, :] = c0                        # bias row rides the pad node
    bt = np.ascontiguousarray(
        btm.reshape(4, 128, N_LEAVES).transpose(1, 0, 2)
    ).astype(bf)                                # [128, 4, 512] bf16
    ct = np.ascontiguousarray(C.T.astype(np.float32)).astype(bf)  # [128, 512]
    return w1t.astype(bf), b1c, bt, ct


def prep_core_inputs(x, W1, b1, W2, leaf_actions):
    """Full host-side prep: per-core input dicts (shard + transpose + cast)."""
    bf = ml_dtypes.bfloat16
    x = np.ascontiguousarray(np.asarray(x, np.float32))
    assert x.shape == (B, IN_DIM)
    w1t, b1c, bt, ct = _prep_weights(W1, b1, W2, leaf_actions)
    xt = np.ascontiguousarray(
        x.reshape(N_CORES, B_SHARD, IN_DIM).transpose(0, 2, 1)
    ).astype(bf)                                # [8, 128, B_SHARD] bf16
    return [
        {"xt": xt[i], "w1t": w1t, "b1c": b1c, "bt": bt, "ct": ct}
        for i in range(N_CORES)
    ]


_runner = None  # (jitted shard_map fn, in_names, sharding, zeros)


def _make_runner(nc):
    """Jitted shard_map wrapper around the bass_exec custom call (mirrors
    bass2jax.run_bass_via_pjrt's multi-core path, but reusable across calls
    so the NEFF is compiled once per process)."""
    import jax
    import numpy as _np
    from jax.sharding import Mesh, PartitionSpec, NamedSharding
    from jax.experimental.shard_map import shard_map
    from concourse import bass2jax, mybir

    bass2jax.install_neuronx_cc_hook()
    partition_name = nc.partition_id_tensor.name if nc.partition_id_tensor else None
    in_names, out_names, out_avals, zero_shapes = [], [], [], []
    for alloc in nc.m.functions[0].allocations:
        if not isinstance(alloc, mybir.MemoryLocationSet):
            continue
        name = alloc.memorylocations[0].name
        if alloc.kind == "ExternalInput":
            if name != partition_name:
                in_names.append(name)
        elif alloc.kind == "ExternalOutput":
            shape = tuple(alloc.tensor_shape)
            dtype = mybir.dt.np(alloc.dtype)
            out_names.append(name)
            out_avals.append(jax.core.ShapedArray(shape, dtype))
            zero_shapes.append((shape, dtype))
    n_params = len(in_names)
    all_in_names = in_names + out_names + ([partition_name] if partition_name else [])

    def _body(*args):
        operands = list(args)
        if partition_name is not None:
            operands.append(bass2jax.partition_id_tensor())
        return tuple(bass2jax._bass_exec_p.bind(
            *operands, out_avals=tuple(out_avals), in_names=tuple(all_in_names),
            out_names=tuple(out_names), lowering_input_output_aliases=(),
            sim_require_finite=True, sim_require_nnan=True, nc=nc))

    mesh = Mesh(_np.asarray(jax.devices()[:N_CORES]), ("core",))
    spec = PartitionSpec("core")
    n_outs = len(out_names)
    fn = jax.jit(
        shard_map(_body, mesh=mesh, in_specs=(spec,) * (n_params + n_outs),
                  out_specs=(spec,) * n_outs, check_rep=False),
        keep_unused=True)
    sh = NamedSharding(mesh, spec)
    zeros = tuple(
        jax.device_put(_np.zeros((N_CORES * s[0], *s[1:]), d), sh)
        for s, d in zero_shapes)
    return fn, in_names, sh, zeros


def kernel(x, W1, b1, W2, leaf_actions):
    global _compiled, _runner
    import jax

    in_maps = prep_core_inputs(x, W1, b1, W2, leaf_actions)

    if _compiled is None:
        _compiled = _build_nc()
    if _runner is None:
        _runner = _make_runner(_compiled)
    fn, in_names, sh, zeros = _runner

    dev_in = [
        jax.device_put(
            np.concatenate([np.asarray(m[nm]) for m in in_maps], axis=0), sh)
        for nm in in_names
    ]
    out = fn(*dev_in, *zeros)
    return np.asarray(out[0])


# revision 8
# speedup vs baseline: 2.6263x; 2.6263x over previous
"""DTNetv0 forward kernel for 8 Trainium2 NeuronCores.

Computes, for x [B,128], W1 [511,128], b1 [511], W2 [512,1022],
leaf_actions [512] (32 leaves per each of 16 actions):

    h = x @ W1.T + b1
    z = [relu(h), relu(-h)]
    y = z @ W2.T
    pooled[b,a] = max over leaves l with action a of y[b,l]
    out = softmax(pooled, axis=-1)

Sharding: pure data parallelism — batch split 8 ways, weights replicated.

Algebraic fold (the key PE-cycle saver): with W2 = [W2a | W2b] split at the
relu(h)/relu(-h) boundary and relu(h) = (h+|h|)/2, relu(-h) = (|h|-h)/2,

    y = h @ A.T + |h| @ B.T,   A = (W2a - W2b)/2,  B = (W2a + W2b)/2
      = x @ C.T + |h| @ B.T + c0,   C = A @ W1,  c0 = A @ b1

so the 1022-wide second contraction becomes a 511-wide one (|h| @ B.T) plus
a 128-wide one (x @ C.T). c0 rides for free through the padded node row:
w1t column 511 is zero and b1 pad is 1.0, so |h|[511] == 1 exactly, and
B.T row 511 holds c0.

ALL matmul operands are bf16: measured on this silicon, bf16 streams at
1 col/cycle with stationary loads fully hidden, while float32r streams at
~1.5 cy/col — bf16 cuts PE time by a third at rel-err 4.4e-3 (gate 2e-2).
x is pre-transposed AND pre-cast to bf16 on the host, halving its DMA.

Per 512-row batch tile, on device:
    hT [512nodes,512b] linear1: 4 bf16 matmuls (W1T stationary, xt moving)
    aT [512, 512b]     Abs(h + b1) on ACT, PSUM -> bf16 SBUF
    y  [128b, 512lv]   linear2 BATCH-MAJOR per 128-batch subtile: 1 matmul
                       x-chunk stationary vs C.T moving, then 4 accumulating
                       matmuls a-chunk stationary vs B.T moving; the two
                       subtiles of a pair land in one 2-bank PSUM tile
    pooled [128b, 16]  leaves are host-permuted ACTION-MAJOR (slot a*32+j),
                       so the segment max is ONE DVE reduce per subtile pair
                       with contiguous 32-element runs (strided reads
                       measured ~2x slower on DVE)
    out                softmax without max-subtraction (|pooled| < 16 so exp
                       is safe in fp32): one ACT Exp [128,4,16], then a
                       DVE-local chain sum -> reciprocal -> stride-0
                       broadcast multiply (no cross-engine ping-pong)

x streams in 8-tile DMA granules (4 dma_starts/pass instead of 32),
double-buffered one granule ahead; the whole pass is ~2.0k instructions so
even an 8x-unrolled NEFF stays under the ~20k instruction-fetch stall wall.

Three-stage software pipeline in emission order: front_a (x -> aT) runs
three tiles ahead of front_b (linear2 + pooled reduce), and the softmax
tail trails one tile behind. PSUM is split 4+4: four banks for h tiles,
two 2-bank rotating tiles for y. Steady state 12288 PE cycles/tile =
5.12 us, PE-bound (bf16 peak); ACT <= ~4 us, DVE <= ~3.5 us, x-DMA
~0.4 us ride underneath.
"""

import numpy as np
import ml_dtypes

B, IN_DIM, N_NODES, N_LEAVES, N_ACTIONS = 131072, 128, 511, 512, 16
N_CORES = 8
B_SHARD = B // N_CORES          # 16384 rows per core
B_TILE = 512                    # batch columns per tile (one PSUM bank of fp32)
N_TILES = B_SHARD // B_TILE     # 32
NODES_P = 512                   # nodes padded 511 -> 512 (4 chunks of 128)
PER_ACTION = N_LEAVES // N_ACTIONS  # 32 leaves per action

_compiled = None  # traced+compiled Bass module cache (one per process)


def _build_nc(n_passes=1):
    import concourse.tile as tile
    from concourse import bacc, bass, mybir
    from contextlib import ExitStack

    fp32 = mybir.dt.float32
    bf16 = mybir.dt.bfloat16
    AF = mybir.ActivationFunctionType

    nc = bacc.Bacc()
    xt_h = nc.declare_dram_parameter("xt", [IN_DIM, B_SHARD], bf16, isOutput=False)
    w1t_h = nc.declare_dram_parameter("w1t", [IN_DIM, NODES_P], bf16, isOutput=False)
    b1c_h = nc.declare_dram_parameter("b1c", [128, 4], fp32, isOutput=False)
    bt_h = nc.declare_dram_parameter("bt", [128, 4, N_LEAVES], bf16, isOutput=False)
    ct_h = nc.declare_dram_parameter("ct", [IN_DIM, N_LEAVES], bf16, isOutput=False)
    out_h = nc.declare_dram_parameter("out", [B_SHARD, N_ACTIONS], fp32, isOutput=True)

    LEAD = 3          # front_a runs this many tiles ahead of front_b
    G_TILES = 8       # batch tiles per x DMA granule
    N_GRAN = N_TILES // G_TILES

    with tile.TileContext(nc) as tc, ExitStack() as ctx:
        consts = ctx.enter_context(tc.tile_pool(name="consts", bufs=1))
        xin = ctx.enter_context(tc.tile_pool(name="xin", bufs=2))
        ap = ctx.enter_context(tc.tile_pool(name="ap", bufs=4))
        sm = ctx.enter_context(tc.tile_pool(name="sm", bufs=2))
        psA = ctx.enter_context(tc.tile_pool(name="psA", bufs=4, space="PSUM"))
        # psY tiles span TWO PSUM banks so the pooled segment-max runs as one
        # DVE reduce per subtile PAIR (fewer, larger DVE instructions)
        psY = ctx.enter_context(tc.tile_pool(name="psY", bufs=2, space="PSUM"))

        # x streams in 8-tile granules (8 KiB/partition each): 4 dma_starts
        # per pass instead of 32, double-buffered one granule ahead.
        granules = {}

        def load_gran(g):
            gx = xin.tile([128, G_TILES * B_TILE], bf16, tag="xg")
            lo = g * G_TILES * B_TILE
            nc.sync.dma_start(out=gx, in_=xt_h[:, lo : lo + G_TILES * B_TILE])
            granules[g] = gx

        load_gran(0)
        b1_sb = consts.tile([128, 4], fp32)
        nc.sync.dma_start(out=b1_sb, in_=b1c_h[:, :])
        w1t_sb = consts.tile([128, NODES_P], bf16)
        nc.sync.dma_start(out=w1t_sb, in_=w1t_h[:, :])
        ct_sb = consts.tile([128, N_LEAVES], bf16)
        nc.sync.dma_start(out=ct_sb, in_=ct_h[:, :])
        # the big bt DMA rides the Activation HWDGE queue so granule loads
        # (SP queue) are not stuck behind it
        bt_sb = consts.tile([128, 4, N_LEAVES], bf16)
        nc.scalar.dma_start(out=bt_sb, in_=bt_h[:, :, :])

        def front_a(t, gi, total):
            rows = slice(t * B_TILE, (t + 1) * B_TILE)
            g, r = divmod(t, G_TILES)
            # prefetch the next granule once all readers of the granule two
            # slots back have been emitted (r == LEAD aligns exactly)
            if r == LEAD and (gi - LEAD + G_TILES) < total:
                load_gran((g + 1) % N_GRAN)
            x_sb = granules[g][:, r * B_TILE : (r + 1) * B_TILE]

            # ---- linear1 + fused bias/abs into aT [128, 4, 512] bf16 ----
            a_sb = ap.tile([128, 4, B_TILE], bf16, tag="a")
            for c in range(4):
                h_ps = psA.tile([128, B_TILE], fp32, tag="h")
                nc.tensor.matmul(
                    h_ps,
                    lhsT=w1t_sb[:, c * 128 : (c + 1) * 128],
                    rhs=x_sb,
                    start=True,
                    stop=True,
                )
                nc.scalar.activation(
                    out=a_sb[:, c, :], in_=h_ps, func=AF.Abs,
                    bias=b1_sb[:, c : c + 1], scale=1.0,
                )
            return rows, x_sb, a_sb

        def front_b(rows, x_sb, a_sb, last=False):
            # ---- linear2, batch-major: y_s [128 batch-sub, 512 leaves] ----
            pl = sm.tile([128, 4, N_ACTIONS], fp32, tag="pl")
            for g in range(2):          # subtile pairs (two PSUM banks each)
                y_ps = psY.tile([128, 2, B_TILE], fp32, tag="y")
                for i in range(2):
                    s = 2 * g + i
                    nc.tensor.matmul(
                        y_ps[:, i, :],
                        lhsT=x_sb[:, s * 128 : (s + 1) * 128],
                        rhs=ct_sb,
                        start=True,
                        stop=False,
                    )
                    for c in range(4):
                        nc.tensor.matmul(
                            y_ps[:, i, :],
                            lhsT=a_sb[:, c, s * 128 : (s + 1) * 128],
                            rhs=bt_sb[:, c, :],
                            start=False,
                            stop=(c == 3),
                        )
                # leaves are action-major (slot a*32 + j): contiguous reduce
                # over both banks in one DVE instruction
                nc.vector.tensor_reduce(
                    out=pl[:, 2 * g : 2 * g + 2, :],
                    in_=y_ps.rearrange("p i (a j) -> p i a j", a=N_ACTIONS),
                    axis=mybir.AxisListType.X,
                    op=mybir.AluOpType.max,
                )
            if last:
                tail(rows, pl)
                return None
            return rows, pl

        def tail(rows, pl):
            # ---- softmax, batch-major [128, 4, 16], no max-subtraction
            # (|pooled| is small enough that exp is safe in fp32).
            # One big ACT Exp, then a DVE-local sum -> reciprocal ->
            # broadcast-multiply chain (no cross-engine ping-pong). ----
            e = sm.tile([128, 4, N_ACTIONS], fp32, tag="e")
            nc.scalar.activation(out=e, in_=pl, func=AF.Exp, scale=1.0)
            ssum = sm.tile([128, 4], fp32, tag="ssum")
            nc.vector.tensor_reduce(
                out=ssum, in_=e, axis=mybir.AxisListType.X,
                op=mybir.AluOpType.add,
            )
            rcp = sm.tile([128, 4], fp32, tag="rcp")
            nc.vector.reciprocal(rcp, ssum)
            o = sm.tile([128, 4, N_ACTIONS], fp32, tag="o")
            rcp_ap = rcp[:, :]
            rcp_bc = bass.AP(rcp_ap.tensor, rcp_ap.offset,
                             rcp_ap.ap + [[0, N_ACTIONS]])
            nc.vector.tensor_tensor(out=o, in0=e, in1=rcp_bc,
                                    op=mybir.AluOpType.mult)
            nc.sync.dma_start(
                out=out_h[rows, :].rearrange("(s p) a -> p s a", p=128), in_=o
            )

        back = tail

        # 3-deep software pipeline: front_a (x -> aT) runs three tiles ahead
        # of front_b (linear2 + pooled reduce); back trails one tile behind.
        total = N_TILES * n_passes
        fa = []
        for t in range(min(LEAD, total)):
            fa.append(front_a(t, t, total))
        pending = None
        for i in range(total):
            cur = front_b(*fa.pop(0), last=(i == total - 1))
            if i + LEAD < total:
                fa.append(front_a((i + LEAD) % N_TILES, i + LEAD, total))
            if pending is not None:
                back(*pending)
            pending = cur
        if pending is not None:
            back(*pending)

    nc.compile()
    return nc


def _prep_weights(W1, b1, W2, leaf_actions):
    """Host-side weight prep: fold the linear half of the relu pair into x
    (C = A@W1, c0 = A@b1) and keep only the |h| half (B) at full width.
    Leaves are permuted ACTION-MAJOR: slot a*32+j holds the j-th leaf of
    action a, so the on-device segment-max reads contiguous runs."""
    W1 = np.asarray(W1, np.float64)
    b1 = np.asarray(b1, np.float64)
    W2 = np.asarray(W2, np.float64)
    bf = ml_dtypes.bfloat16

    la = np.asarray(leaf_actions).astype(np.int64)
    perm = np.empty(N_LEAVES, np.int64)
    for a in range(N_ACTIONS):
        (grp,) = np.nonzero(la == a)
        assert len(grp) == PER_ACTION, "kernel assumes 32 leaves per action"
        perm[a * PER_ACTION + np.arange(PER_ACTION)] = grp

    W2p = W2[perm]                              # [512, 1022] leaf-permuted
    Am = (W2p[:, :N_NODES] - W2p[:, N_NODES:]) * 0.5   # [512, 511]
    Bm = (W2p[:, :N_NODES] + W2p[:, N_NODES:]) * 0.5   # [512, 511]
    C = Am @ W1                                 # [512, 128]
    c0 = Am @ b1                                # [512]

    w1t = np.zeros((IN_DIM, NODES_P), np.float32)
    w1t[:, :N_NODES] = W1.T                     # col 511 stays zero
    b1c = np.zeros((4, 128), np.float32)
    b1c.reshape(-1)[:N_NODES] = b1
    b1c.reshape(-1)[N_NODES] = 1.0              # pad node: |h|[511] == 1
    b1c = np.ascontiguousarray(b1c.T)           # [128, 4]

    btm = np.zeros((NODES_P, N_LEAVES), np.float32)
    btm[:N_NODES, :] = Bm.T
    btm[N_NODES, :] = c0                        # bias row rides the pad node
    bt = np.ascontiguousarray(
        btm.reshape(4, 128, N_LEAVES).transpose(1, 0, 2)
    ).astype(bf)                                # [128, 4, 512] bf16
    ct = np.ascontiguousarray(C.T.astype(np.float32)).astype(bf)  # [128, 512]
    return w1t.astype(bf), b1c, bt, ct


def prep_core_inputs(x, W1, b1, W2, leaf_actions):
    """Full host-side prep: per-core input dicts (shard + transpose + cast)."""
    bf = ml_dtypes.bfloat16
    x = np.ascontiguousarray(np.asarray(x, np.float32))
    assert x.shape == (B, IN_DIM)
    w1t, b1c, bt, ct = _prep_weights(W1, b1, W2, leaf_actions)
    xt = np.ascontiguousarray(
        x.reshape(N_CORES, B_SHARD, IN_DIM).transpose(0, 2, 1)
    ).astype(bf)                                # [8, 128, B_SHARD] bf16
    return [
        {"xt": xt[i], "w1t": w1t, "b1c": b1c, "bt": bt, "ct": ct}
        for i in range(N_CORES)
    ]


_runner = None  # (jitted shard_map fn, in_names, sharding, zeros)


def _make_runner(nc):
    """Jitted shard_map wrapper around the bass_exec custom call (mirrors
    bass2jax.run_bass_via_pjrt's multi-core path, but reusable across calls
    so the NEFF is compiled once per process)."""
    import jax
    import numpy as _np
    from jax.sharding import Mesh, PartitionSpec, NamedSharding
    from jax.experimental.shard_map import shard_map
    from concourse import bass2jax, mybir

    bass2jax.install_neuronx_cc_hook()
    partition_name = nc.partition_id_tensor.name if nc.partition_id_tensor else None
    in_names, out_names, out_avals, zero_shapes = [], [], [], []
    for alloc in nc.m.functions[0].allocations:
        if not isinstance(alloc, mybir.MemoryLocationSet):
            continue
        name = alloc.memorylocations[0].name
        if alloc.kind == "ExternalInput":
            if name != partition_name:
                in_names.append(name)
        elif alloc.kind == "ExternalOutput":
            shape = tuple(alloc.tensor_shape)
            dtype = mybir.dt.np(alloc.dtype)
            out_names.append(name)
            out_avals.append(jax.core.ShapedArray(shape, dtype))
            zero_shapes.append((shape, dtype))
    n_params = len(in_names)
    all_in_names = in_names + out_names + ([partition_name] if partition_name else [])

    def _body(*args):
        operands = list(args)
        if partition_name is not None:
            operands.append(bass2jax.partition_id_tensor())
        return tuple(bass2jax._bass_exec_p.bind(
            *operands, out_avals=tuple(out_avals), in_names=tuple(all_in_names),
            out_names=tuple(out_names), lowering_input_output_aliases=(),
            sim_require_finite=True, sim_require_nnan=True, nc=nc))

    mesh = Mesh(_np.asarray(jax.devices()[:N_CORES]), ("core",))
    spec = PartitionSpec("core")
    n_outs = len(out_names)
    fn = jax.jit(
        shard_map(_body, mesh=mesh, in_specs=(spec,) * (n_params + n_outs),
                  out_specs=(spec,) * n_outs, check_rep=False),
        keep_unused=True)
    sh = NamedSharding(mesh, spec)
    zeros = tuple(
        jax.device_put(_np.zeros((N_CORES * s[0], *s[1:]), d), sh)
        for s, d in zero_shapes)
    return fn, in_names, sh, zeros


def kernel(x, W1, b1, W2, leaf_actions):
    global _compiled, _runner
    import jax

    in_maps = prep_core_inputs(x, W1, b1, W2, leaf_actions)

    if _compiled is None:
        _compiled = _build_nc()
    if _runner is None:
        _runner = _make_runner(_compiled)
    fn, in_names, sh, zeros = _runner

    dev_in = [
        jax.device_put(
            np.concatenate([np.asarray(m[nm]) for m in in_maps], axis=0), sh)
        for nm in in_names
    ]
    out = fn(*dev_in, *zeros)
    return np.asarray(out[0])
